# revision 13
# baseline (speedup 1.0000x reference)
"""CapsuleLayer dynamic-routing kernel for 8 TRN2 NeuronCores.

Math (per reference):
  priors[c,b,r,o] = sum_i x[b,r,i] * W[c,r,i,o]      b=256, r=1152, i=8, c=10, o=16
  3 routing iterations of softmax(logits over r) -> squash -> logit update.

Sharding: data-parallel over b (8 cores x 32 batch). W k-sharded on the wire
(1/8 per core) and AllGathered on-device over NeuronLink; the block-diagonal
x operand (4x the size of x) is built on-device from the compact x2dt with a
memset + 16 strided copies per DMA piece. End-to-end wall time here is
dominated by host->device transfer through the axon tunnel (~40 MB/s), so the
wire carries only x (f16, 4.7MB), W (f16, 2.95MB total, sharded) and two tiny
f16 0/1 mask matrices.

Per-core layout: partition p = 4*b + j where j = r mod 4; r = 4*g + j, g in [0,288).
priors stored in SBUF as fp16 [128, g=288, c=10, o=16].
priors computed by 288 small matmuls: stationary lhsT = block-diag x
[(j,i)=32, (b,j)=128], moving rhs = W slice [(j,i)=32, (c,o)=160], PSUM out
[(b,j)=128, (c,o)=160]. Matmul inputs quantized to fp16 (rel err ~4e-3).
Iteration-0 mean over r via a dense K=9216 accumulated matmul (uniform
softmax). Cross-partition j-sums / b-broadcasts via tiny constant matmuls
(S = sum4, E = expand4). Reductions over o / g on DVE with strided APs; exp on
ACT in chunks (no max-subtraction: |logits| <~ 70 fits fp32 range).
"""

import numpy as np

B_FULL, R, I, C, O = 256, 1152, 8, 10, 16
NCORES = 8
B = B_FULL // NCORES          # 32 batch per core
G = R // 4                    # 288 groups of 4 r-values
K72 = R // 16                 # 72 chunks of 16 r (4 groups stacked)
CO = C * O                    # 160
GCHUNK = 18                   # routing g-chunk
NCHUNK = G // GCHUNK          # 16
SLAB = 3                      # priors groups per PSUM bank-slab
DMA_SPLIT = 8                 # k-chunks per input DMA piece
KSH = K72 // NCORES           # 9 k-chunks of W per core on the wire

_CACHE = {}


def _build_bass(stage=5):
    import concourse.bass as bass
    import concourse.bacc as bacc
    import concourse.mybir as mybir
    from concourse.tile import TileContext
    from contextlib import ExitStack

    f32, f16 = mybir.dt.float32, mybir.dt.float16
    Act = mybir.ActivationFunctionType
    AX, ADD = mybir.AxisListType.X, mybir.AluOpType.add

    nc = bacc.Bacc("TRN2", target_bir_lowering=False, debug=False,
                   enable_asserts=False, num_devices=NCORES)

    x2dt_d = nc.dram_tensor("x2dt", [128, K72 * B], f16, kind="ExternalInput")
    wsh_d = nc.dram_tensor("wsh", [KSH, 128, CO], f16, kind="ExternalInput")
    c_d = nc.dram_tensor("consts", [128, B + 4], f16, kind="ExternalInput")
    out_d = nc.dram_tensor("out", [B, CO], f16, kind="ExternalOutput")

    with ExitStack() as ctx:
        tc = ctx.enter_context(TileContext(nc))
        pers = ctx.enter_context(tc.tile_pool(name="pers", bufs=1))
        pp = ctx.enter_context(tc.tile_pool(name="pp", bufs=4, space="PSUM"))
        sp = ctx.enter_context(tc.tile_pool(name="sp", bufs=1, space="PSUM"))
        rt = ctx.enter_context(tc.tile_pool(name="rt", bufs=2))
        sm = ctx.enter_context(tc.tile_pool(name="sm", bufs=1))
        dram = ctx.enter_context(tc.tile_pool(name="dram", bufs=2, space="DRAM"))

        priors = pers.tile([128, G, C, O], f16)
        logits = pers.tile([128, G, C], f32)
        vexp = pers.tile([128, C, O], f16)
        smat = pers.tile([128, B], f32)
        emat = pers.tile([B, 128], f32)
        sm16 = pers.tile([128, B], f16)
        em16 = pers.tile([B, 128], f16)
        jmask = pers.tile([128, 4], f16)
        vout16 = pers.tile([B, CO], f16)

        nc.sync.dma_start(out=sm16, in_=c_d.ap()[:, 0:B])
        nc.sync.dma_start(out=jmask, in_=c_d.ap()[:, B:B + 4])
        # emat = smat^T, via element-granular swapped-AP DMA from DRAM
        nc.sync.dma_start(out=em16, in_=c_d.ap()[:, 0:B].rearrange("a b -> b a"))
        nc.vector.tensor_copy(out=smat, in_=sm16)
        nc.vector.tensor_copy(out=emat, in_=em16)

        # ---- W: 1/8 shard per core -> AllGather over NeuronLink ----
        wbin = dram.tile([KSH, 128, CO], f16)
        wgat = dram.tile([K72, 128, CO], f16)
        nc.gpsimd.dma_start(wbin[:], wsh_d.ap())
        nc.gpsimd.collective_compute(
            "AllGather", mybir.AluOpType.bypass,
            replica_groups=[list(range(NCORES))],
            ins=[wbin.opt()], outs=[wgat.opt()])

        KC = K72 // DMA_SPLIT  # 9 k per piece
        with tc.tile_pool(name="mmin", bufs=1) as mmin:
            xbl, wbl, x2l = [], [], []
            for d in range(DMA_SPLIT):
                x2 = mmin.tile([128, KC, B], f16, tag=f"x2{d}", name=f"x2{d}")
                nc.sync.dma_start(out=x2, in_=x2dt_d.ap()[:, d * KC * B:(d + 1) * KC * B])
                x2l.append(x2)
                wt = mmin.tile([128, KC, CO], f16, tag=f"wb{d}", name=f"wb{d}")
                for k in range(KC):
                    nc.gpsimd.dma_start(out=wt[:, k, :], in_=wgat[d * KC + k])
                wbl.append(wt)
            for d in range(DMA_SPLIT):
                # block-diag x built on-device: [p=(q,j,i), k, (b,j2)] with
                # x only on the j==j2 diagonal blocks, via one broadcast mul
                xt = mmin.tile([128, KC, 128], f16, tag=f"xb{d}", name=f"xb{d}")
                xv = xt.rearrange("p k (b j) -> p k b j", j=4)
                nc.vector.tensor_mul(
                    xv,
                    x2l[d][:, :, :, None].broadcast_to([128, KC, B, 4]),
                    jmask[:, None, None, :].broadcast_to([128, KC, B, 4]))
                xbl.append(xt)

            # ---- s0 = (1/1152) * sum_r priors : dense K=9216 matmul ----
            s0_ps = sp.tile([B, CO], f32, bufs=1)
            for k in range(K72):
                nc.tensor.matmul(s0_ps, x2l[k // KC][:, k % KC, :], wbl[k // KC][:, k % KC, :],
                                 start=(k == 0), stop=(k == K72 - 1))

            # ---- priors: 288 block-diag matmuls, drain psum->sbuf fp16 ----
            # Slabs keep one row-strip (q) per PSUM bank: concurrent MMs on
            # different row strips must not share a bank (HW crash observed).
            slabs = []
            if stage >= 2:
                for q in range(4):
                    for k0 in range(0, K72, SLAB):
                        slabs.append((q, k0))
            for si, (q, k0) in enumerate(slabs):
                ps = pp.tile([128, SLAB, CO], f32, tag="slab", name=f"slab{si}")
                for u in range(SLAB):
                    k = k0 + u
                    nc.tensor.matmul(
                        ps[:, u, :],
                        xbl[k // KC][32 * q:32 * q + 32, k % KC, :],
                        wbl[k // KC][32 * q:32 * q + 32, k % KC, :],
                        start=True, stop=True, tile_position=(32 * q, 0))
                dst = priors.rearrange("p (k q) c o -> p q k (c o)", q=4)[:, q, k0:k0 + SLAB, :]
                if si % 2 == 0:
                    nc.scalar.copy(out=dst, in_=ps)
                else:
                    nc.vector.tensor_copy(out=dst, in_=ps)

        # scratch [B, *] f32 slices for squash / normalize temps
        scr = pers.tile([B, 1024], f32)
        s_sb = scr[:, 0:160].rearrange("b (c o) -> b c o", c=C)
        ssq = scr[:, 160:320].rearrange("b (c o) -> b c o", c=C)
        v_sb = scr[:, 320:480].rearrange("b (c o) -> b c o", c=C)
        sq = scr[:, 480:490]
        sqs = scr[:, 490:500]
        den = scr[:, 500:510]
        rden = scr[:, 510:520]
        fsc = scr[:, 520:530]
        rz = scr[:, 540:550]

        sparts = pers.tile([128, NCHUNK, C, O], f32)
        zparts = pers.tile([128, NCHUNK, C], f32)

        def squash_from_s(scale_extra):
            """v_sb = squash(scale_extra * s_sb)."""
            sc2 = scale_extra * scale_extra
            nc.vector.tensor_mul(ssq, s_sb, s_sb)
            nc.vector.tensor_reduce(sq, ssq, axis=AX, op=ADD)
            nc.scalar.activation(sqs, sq, func=Act.Sqrt, scale=sc2)
            nc.scalar.mul(out=den, in_=sq, mul=sc2)
            nc.scalar.add(out=den, in_=den, add=1.0)
            nc.vector.reciprocal(rden, den)
            nc.vector.tensor_mul(fsc, sqs, rden)
            if scale_extra != 1.0:
                nc.scalar.mul(out=fsc, in_=fsc, mul=scale_extra)
            nc.vector.tensor_mul(v_sb, s_sb, fsc[:, :, None].broadcast_to([B, C, O]))

        def expand_v():
            """vexp [128, C, O] f16 = replicate v_sb over j."""
            vps = sp.tile([128, CO], f32, tag="vps", bufs=1, name="vps")
            nc.tensor.matmul(vps, emat, v_sb.rearrange("b c o -> b (c o)"),
                             start=True, stop=True)
            nc.scalar.copy(out=vexp.rearrange("p c o -> p (c o)"), in_=vps)

        def delta_acc(first):
            """logits (+)= sum_o priors * vexp. o-reduction as in-place fp16
            halving tree (TT-add at 2x beats tensor_reduce's 1x cap)."""
            for h in range(NCHUNK):
                g0 = h * GCHUNK
                tmp = rt.tile([128, GCHUNK, C, O], f16, tag="dtmp", name=f"dtmp{h}")
                nc.vector.tensor_mul(
                    tmp, priors[:, g0:g0 + GCHUNK],
                    vexp[:, None, :, :].broadcast_to([128, GCHUNK, C, O]))
                for w in (8, 4, 2):
                    nc.vector.tensor_add(tmp[:, :, :, 0:w], tmp[:, :, :, 0:w],
                                         tmp[:, :, :, w:2 * w])
                if first:
                    nc.vector.tensor_add(logits[:, g0:g0 + GCHUNK],
                                         tmp[:, :, :, 0], tmp[:, :, :, 1])
                else:
                    dpart = rt.tile([128, GCHUNK, C], f32, tag="dpart", name=f"dpart{h}")
                    nc.vector.tensor_add(dpart, tmp[:, :, :, 0], tmp[:, :, :, 1])
                    nc.vector.tensor_add(logits[:, g0:g0 + GCHUNK],
                                         logits[:, g0:g0 + GCHUNK], dpart)

        def s_iter(tag):
            """writes s_sb = softmax(logits)-weighted sum of priors (normalized)."""
            for h in range(NCHUNK):
                g0 = h * GCHUNK
                ec = rt.tile([128, GCHUNK, C], f32, tag="ec", name=f"ec{h}")
                nc.scalar.activation(ec.rearrange("p g c -> p (g c)"),
                                     logits[:, g0:g0 + GCHUNK].rearrange("p g c -> p (g c)"),
                                     func=Act.Exp)
                stmp = rt.tile([128, GCHUNK, C, O], f32, tag="stmp", name=f"stmp{h}")
                nc.vector.tensor_mul(
                    stmp, priors[:, g0:g0 + GCHUNK],
                    ec[:, :, :, None].broadcast_to([128, GCHUNK, C, O]))
                nc.vector.tensor_reduce(sparts[:, h], stmp.rearrange("p g c o -> p c o g"),
                                        axis=AX, op=ADD)
                nc.vector.tensor_reduce(zparts[:, h], ec.rearrange("p g c -> p c g"),
                                        axis=AX, op=ADD)
            sfin = sm.tile([128, C, O], f32, tag="sfin", name="sfin")
            nc.vector.tensor_reduce(sfin, sparts.rearrange("p h c o -> p c o h"),
                                    axis=AX, op=ADD)
            zfin = sm.tile([128, C], f32, tag="zfin", name="zfin")
            nc.vector.tensor_reduce(zfin, zparts.rearrange("p h c -> p c h"),
                                    axis=AX, op=ADD)
            sj_ps = sp.tile([B, CO], f32, tag="sj", bufs=1, name=f"sj{tag}")
            nc.tensor.matmul(sj_ps, smat, sfin.rearrange("p c o -> p (c o)"),
                             start=True, stop=True)
            zj_ps = sp.tile([B, C], f32, tag="zj", bufs=1, name=f"zj{tag}")
            nc.tensor.matmul(zj_ps, smat, zfin, start=True, stop=True)
            nc.vector.reciprocal(rz, zj_ps)
            nc.vector.tensor_mul(s_sb, sj_ps.rearrange("b (c o) -> b c o", c=C),
                                 rz[:, :, None].broadcast_to([B, C, O]))

        if stage >= 1:
            pass
        # ---- iteration 0 ----
        nc.vector.tensor_copy(out=s_sb, in_=s0_ps.rearrange("b (c o) -> b c o", c=C))
        squash_from_s(1.0 / R)
        if stage >= 3:
            expand_v()
            delta_acc(first=True)
        if stage >= 4:
            # ---- iteration 1 ----
            s_iter("1")
            squash_from_s(1.0)
        if stage >= 5:
            expand_v()
            delta_acc(first=False)
            # ---- iteration 2 ----
            s_iter("2")
            squash_from_s(1.0)
        nc.vector.tensor_copy(out=vout16, in_=v_sb.rearrange("b c o -> b (c o)"))
        nc.sync.dma_start(out=out_d.ap(), in_=vout16)

    nc.finalize()
    return nc


def _prep_inputs(x, route_weights):
    """Global (concat-over-cores) wire tensors, f16."""
    x = np.asarray(x, dtype=np.float32)
    W = np.asarray(route_weights, dtype=np.float32)
    # xt[m, k, q, j, i, b] = x[32m+b, 16k+4q+j, i]
    xt = x.reshape(NCORES, B, K72, 4, 4, I).transpose(0, 2, 3, 4, 5, 1).astype(np.float16)
    x2dt = np.ascontiguousarray(
        xt.reshape(NCORES, K72, 128, B).transpose(0, 2, 1, 3)).reshape(NCORES * 128, K72 * B)
    # wsh[m, ksh, p=(q,j,i), (c,o)]: k-major so AllGather slots concat to [k, p, co]
    wk = W.reshape(C, K72, 4, 4, I, O).transpose(1, 2, 3, 4, 0, 5).astype(np.float16)
    wsh = wk.reshape(NCORES * KSH, 128, CO)
    # consts[:, 0:B] = smat (smat[4b+j, b] = 1), consts[:, B:B+4] = jmask
    consts = np.zeros((128, B + 4), dtype=np.float16)
    for b in range(B):
        consts[4 * b:4 * b + 4, b] = 1.0
    for p in range(128):
        consts[p, B + (p // 8) % 4] = 1.0
    return {
        "x2dt": x2dt,
        "wsh": wsh,
        "consts": np.tile(consts, (NCORES, 1)),
    }


def _build_runner():
    """Compile the Bass module once and return a cached jitted SPMD callable.

    Mirrors what bass_utils.run_bass_kernel_spmd -> bass2jax.run_bass_via_pjrt
    does under axon, but constructs the jax.jit(shard_map(...)) closure ONCE so
    repeat calls skip re-tracing/lowering (~0.1s/call through the tunnel).
    """
    import jax
    import jax.numpy as jnp
    from jax.sharding import Mesh, PartitionSpec
    from jax.experimental.shard_map import shard_map
    import concourse.mybir as mybir
    from concourse import bass2jax

    nc = _build_bass()
    bass2jax.install_neuronx_cc_hook()

    partition_name = nc.partition_id_tensor.name if nc.partition_id_tensor else None
    in_names, out_names, out_avals = [], [], []
    for alloc in nc.m.functions[0].allocations:
        if not isinstance(alloc, mybir.MemoryLocationSet):
            continue
        name = alloc.memorylocations[0].name
        if alloc.kind == "ExternalInput":
            if name != partition_name:
                in_names.append(name)
        elif alloc.kind == "ExternalOutput":
            out_names.append(name)
            out_avals.append(jax.core.ShapedArray(
                tuple(alloc.tensor_shape), mybir.dt.np(alloc.dtype)))
    n_params = len(in_names)
    all_names = list(in_names) + out_names + ([partition_name] if partition_name else [])
    donate = tuple(range(n_params, n_params + len(out_names)))

    def _body(*args):
        operands = list(args)
        if partition_name is not None:
            operands.append(bass2jax.partition_id_tensor())
        outs = bass2jax._bass_exec_p.bind(
            *operands,
            out_avals=tuple(out_avals),
            in_names=tuple(all_names),
            out_names=tuple(out_names),
            lowering_input_output_aliases=(),
            sim_require_finite=True,
            sim_require_nnan=True,
            nc=nc,
        )
        return tuple(outs)

    mesh = Mesh(np.asarray(jax.devices()[:NCORES]), ("core",))
    nio = n_params + len(out_names)
    sharded = jax.jit(
        shard_map(_body, mesh=mesh, in_specs=(PartitionSpec("core"),) * nio,
                  out_specs=(PartitionSpec("core"),) * len(out_names),
                  check_rep=False),
        donate_argnums=donate, keep_unused=True)

    out_shapes = [(NCORES * a.shape[0], *a.shape[1:]) for a in out_avals]
    out_dtypes = [a.dtype for a in out_avals]

    def run(feed):
        args = [feed[n] for n in in_names]
        args += [np.zeros(s, d) for s, d in zip(out_shapes, out_dtypes)]
        outs = sharded(*args)
        return {n: np.asarray(o) for n, o in zip(out_names, outs)}

    return run


def kernel(x, route_weights):
    if "runner" not in _CACHE:
        _CACHE["runner"] = _build_runner()
    runner = _CACHE["runner"]

    feed = _prep_inputs(x, route_weights)
    import time as _time
    _t0 = _time.time()
    outs = runner(feed)
    _CACHE["last_run_wall_s"] = _time.time() - _t0
    _CACHE["last_results"] = None
    return outs["out"].astype(np.float32).reshape(B_FULL, C, O)
